# revision 1
# baseline (speedup 1.0000x reference)
"""Trainium2 Bass kernel for nn_Euclidean (retrieval_knn).

Computes out[b, c] = -mean_f (x[b, f] - w[c, f])^2 for x [16384, 2048] f32,
w [1000, 2048] f32, via the algebraic expansion

    out = (2/F) * (x @ w.T) - ||x_b||^2 / F - ||w_c||^2 / F

Sharding: data-parallel over the batch dim across 8 NeuronCores; w replicated.

Per-core dataflow, software-pipelined with lag 8:
  - rounds 0..7:  w-chunk setup + x-chunk prep (DMA-heavy; PE transposes)
  - rounds 8..15: x-chunk prep + DoubleRow GEMM of chunk r-8
  - rounds 16..23: remaining GEMMs (PE-dense tail)

Per-tile prep: HWDGE DMA fp32 tile -> DVE bf16 cast -> DVE fused
tensor_tensor_reduce(bf*bf*scale) producing the row-norm column exactly in
fp32 accum (scale=-1/F for x gives the ACT bias -||x_b||^2/F directly;
scale=-0.5 for w gives -||w_c||^2/2) -> TensorE bf16 transpose-mode
matmuls -> ScalarE Identity evacuation PSUM -> fp8e4 SBUF.

GEMM: 16 DoubleRow fp8 matmuls accumulate x @ w.T per PSUM half.  lhsT/rhs
are 3D APs [128 ki, 2 plane, free] whose planes are adjacent 128-k chunks
(plane stride %16==0 satisfies s3_lw_dual_fp8_restrictions); each
instruction contracts 256 k at ~2 fp8 columns/cycle.  The fp8 quantization
noise only touches the (2/F)*x@w.T term (~1e-3 of the output magnitude).
ScalarE Identity evacuates PSUM with scale=2/F and per-partition bias
-||x_b||^2/F; DVE adds the -||w_c||^2/F row (w2bc, built once via K=1
ones-matmul broadcast).

Walrus encodes at most one semaphore wait per LDWEIGHTS/MM struct: dummy
standalone LDWEIGHTS absorb cross-engine waits ahead of each transpose
group (add_dep_helper keeps them ordered), and _legalize_waits splits any
remaining multi-wait instructions.
"""

import math
import os
import sys

import numpy as np

if "/opt/trn_rl_repo" not in sys.path:
    sys.path.insert(0, "/opt/trn_rl_repo")

N_CORES = 8
B_TOTAL = 16384
F = 2048
C = 1000

_cache = {}
LAST_RESULTS = None


def _legalize_waits(nc):
    """Walrus encodes at most ONE sync-wait per instruction struct, but Tile's
    sem assignment freely attaches several. Split: hoist all but the last wait
    onto standalone EventSemaphore instructions (pure sem-op carriers) placed
    immediately before the over-limit instruction on the same engine queue."""
    import bass_rust
    import concourse.mybir as mybir

    n = 0
    for f in nc.m.functions:
        for bb in f.blocks:
            newlist = []
            for inst in bb.instructions:
                si = inst.sync_info
                if si is not None and len(si.on_wait) > 1:
                    waits = list(si.on_wait)
                    for w in waits[:-1]:
                        ev = mybir.InstEventSemaphore(
                            name=f"waitsplit_{n}", ins=[], outs=[]
                        )
                        ev.engine = inst.engine
                        ev.sync_info = bass_rust.SyncInfo(on_wait=[w], on_update=[])
                        newlist.append(ev)
                        n += 1
                    inst.sync_info = bass_rust.SyncInfo(
                        on_wait=[waits[-1]], on_update=list(si.on_update)
                    )
                newlist.append(inst)
            bb.instructions = newlist
    return n


def _build():
    import concourse.bass as bass
    import concourse.mybir as mybir
    from bass_rust import add_dep_helper
    from concourse.masks import make_identity
    from concourse.tile import TileContext

    P = 128
    KT = F // P                 # 16 contraction chunks of 128
    KD = KT // 2                # 8 DoubleRow plane-pairs of 256
    B = B_TOTAL // N_CORES      # 2048 batch rows per core
    BT = B // P                 # 16 batch chunks
    CP = 1024                   # padded class dim
    CT = CP // P                # 8 class chunks
    KG = 4                      # k-chunks per PSUM transpose group
    f8 = mybir.dt.float8e4
    bdt = mybir.dt.bfloat16
    fdt = mybir.dt.float32
    AF = mybir.ActivationFunctionType
    ALU = mybir.AluOpType
    DR = mybir.MatmulPerfMode.DoubleRow

    nc = bass.Bass()
    x = nc.dram_tensor("x", [B, F], fdt, kind="ExternalInput")
    w = nc.dram_tensor("w", [C, F], fdt, kind="ExternalInput")
    out = nc.dram_tensor("out", [B, C], fdt, kind="ExternalOutput")

    with TileContext(nc) as tc:
        with (
            tc.tile_pool(name="consts", bufs=1) as constp,
            tc.tile_pool(name="wstage", bufs=3) as wp,
            tc.tile_pool(name="xstage", bufs=3) as xp,
            tc.tile_pool(name="evac", bufs=3) as ep,
            tc.tile_pool(name="dram", bufs=1, space="DRAM") as dp,
            tc.tile_pool(name="psum", bufs=2, space="PSUM") as pp,
        ):
            ones_row = constp.tile([1, P], bdt)
            nc.vector.memset(ones_row[:, :], 1.0)
            ident = constp.tile([P, P], bdt)
            make_identity(nc, ident[:, :])

            # Sacrificial transpose: absorbs the one-time identity-readiness
            # wait so later transposes carry only their single data wait.
            pwarm = pp.tile([P, P], bdt, tag="pst", bufs=4)
            nc.tensor.transpose(pwarm[:, :], ident[:, :], ident[:, :])

            wT = constp.tile([P, KT, CP], f8)     # w^T fp8, resident all kernel
            w2neg = constp.tile([1, CP], bdt)     # -||w_c||^2 / 2
            w2row = constp.tile([1, CP], fdt)
            w2d = dp.tile([CP, 1], fdt)
            # all 16 xT tiles stay resident (2KB/partition each)
            xTs = [
                constp.tile([P, KT, P], f8, name=f"xT_{i}") for i in range(BT)
            ]
            negx2s = [
                constp.tile([P, 1], fdt, name=f"negx2_{i}") for i in range(BT)
            ]

            dum_pool = {"prev": None}

            def transpose_evac(bftile, put_evac):
                """16 bf16 transposes in 4 PSUM groups + fp8 evacs."""
                dums = [nc.tensor.ldweights(bftile[:, 0:P])]
                if dum_pool["prev"] is not None:
                    dums.append(nc.tensor.ldweights(dum_pool["prev"]))
                for kg in range(KT // KG):
                    pst = pp.tile([P, KG * P], bdt, tag="pst", bufs=4)
                    for q in range(KG):
                        k = kg * KG + q
                        t = nc.tensor.transpose(
                            pst[:, q * P : (q + 1) * P],
                            bftile[:, k * P : (k + 1) * P],
                            ident[:, :],
                        )
                        if q == 0:
                            for d in dums:
                                add_dep_helper(
                                    t.ins, d.ins, sync=False,
                                    reason="keep wait-absorber LDW before transposes",
                                )
                    put_evac(kg, pst[:, :].rearrange("p (k c) -> p k c", k=KG))
                dum_pool["prev"] = bftile[:, (KT - 1) * P : KT * P]

            def w_setup(j):
                c0 = j * P
                csz = min(P, C - c0)
                w_f32 = wp.tile([P, F], fdt, tag="w_f32", bufs=4)
                nc.sync.dma_start(out=w_f32[:csz, :], in_=w[c0 : c0 + csz, :])
                w_bf = wp.tile([P, F], bdt, tag="w_bf")
                if csz < P:
                    # pad rows feed the transpose below; keep them finite.
                    # (partition starts must be 32-aligned; the copy below
                    # overwrites the real rows inside the padded range)
                    pad_base = (csz // 32) * 32
                    nc.vector.memset(w_bf[pad_base:P, :], 0.0)
                nc.vector.tensor_copy(w_bf[:csz, :], w_f32[:csz, :])
                wsq = wp.tile([P, F], bdt, tag="wsq", bufs=2)
                w2col = wp.tile([P, 1], fdt, tag="w2col")
                nc.scalar.activation(
                    wsq[:csz, :], w_f32[:csz, :], AF.Square,
                    accum_out=w2col[:csz, :],
                )
                nc.sync.dma_start(out=w2d[c0 : c0 + csz, :], in_=w2col[:csz, :])

                def put(kg, src):
                    dst = wT[:, kg * KG : (kg + 1) * KG, c0 : c0 + P]
                    if kg == 0:
                        nc.scalar.activation(dst, src, AF.Identity)
                    else:
                        nc.vector.tensor_copy(dst, src)
                transpose_evac(w_bf, put)

            def x_load(i):
                x_f32 = xp.tile([P, F], fdt, tag="x_f32", bufs=6,
                                name=f"x_f32_{i}")
                nc.scalar.dma_start(out=x_f32[:, :], in_=x[i * P : (i + 1) * P, :])
                return x_f32

            inv_sqrt_f = 1.0 / math.sqrt(F)

            def x_prep(i, x_f32, n_act_evacs):
                x_bf = xp.tile([P, F], bdt, tag="x_bf")
                nc.vector.tensor_copy(x_bf[:, :], x_f32[:, :])
                xsq = xp.tile([P, F], bdt, tag="xsq", bufs=2)
                x2c = xp.tile([P, 1], fdt, tag="x2c", bufs=2)
                # accum_out = sum_f (x/sqrt(F))^2 = ||x_b||^2 / F
                nc.scalar.activation(
                    xsq[:, :], x_f32[:, :], AF.Square,
                    scale=inv_sqrt_f, accum_out=x2c[:, :],
                )
                nc.vector.tensor_scalar_mul(negx2s[i][:, :], x2c[:, :], -1.0)

                def put(kg, src):
                    dst = xTs[i][:, kg * KG : (kg + 1) * KG, :]
                    if kg < n_act_evacs:
                        nc.scalar.activation(dst, src, AF.Identity)
                    else:
                        nc.vector.tensor_copy(dst, src)
                transpose_evac(x_bf, put)

            def gemm(i):
                b0 = i * P
                xT = xTs[i]
                ps_a = pp.tile([P, 512], fdt, tag="ps_a")
                ps_b = pp.tile([P, 512], fdt, tag="ps_b")
                for d in range(KD):
                    nc.tensor.matmul(
                        ps_a[:, :],
                        xT[:, 2 * d : 2 * d + 2, :],
                        wT[:, 2 * d : 2 * d + 2, 0:512],
                        start=(d == 0), stop=(d == KD - 1),
                        perf_mode=DR,
                    )
                for d in range(KD):
                    nc.tensor.matmul(
                        ps_b[:, 0:488],
                        xT[:, 2 * d : 2 * d + 2, :],
                        wT[:, 2 * d : 2 * d + 2, 512:1000],
                        start=(d == 0), stop=(d == KD - 1),
                        perf_mode=DR,
                    )

                o_sb = ep.tile([P, C], fdt, tag="o_sb")
                nc.scalar.activation(
                    o_sb[:, 0:512], ps_a[:, :], AF.Identity,
                    bias=negx2s[i][:, 0:1], scale=2.0 / F,
                )
                nc.scalar.activation(
                    o_sb[:, 512:1000], ps_b[:, 0:488], AF.Identity,
                    bias=negx2s[i][:, 0:1], scale=2.0 / F,
                )
                nc.vector.tensor_add(o_sb[:, 0:C], o_sb[:, 0:C], w2bc[:, 0:C])
                nc.sync.dma_start(out=out[b0 : b0 + P, :], in_=o_sb[:, :])

            # ---- Phase 1: rounds 0..7 -- w setup + x prep ----
            for r in range(CT):
                xf = x_load(r)
                w_setup(r)
                x_prep(r, xf, n_act_evacs=2)

            # w2 gather (DRAM round-trip transposes the column to a row) and
            # the w2bc broadcast; emitted right after the last w chunk so it
            # overlaps the phase-2 pipeline head.
            nc.sync.dma_start(
                out=w2row[0:1, 0:C], in_=w2d[0:C, :].rearrange("c one -> one c")
            )
            nc.scalar.mul(w2neg[0:1, 0:C], w2row[0:1, 0:C], -0.5)
            w2bc = constp.tile([P, CP], fdt)
            w2ps_a = pp.tile([P, 512], fdt, tag="ps_a")
            nc.tensor.matmul(
                w2ps_a[:, :], ones_row[0:1, :], w2neg[0:1, 0:512],
                start=True, stop=True,
            )
            nc.scalar.activation(
                w2bc[:, 0:512], w2ps_a[:, :], AF.Identity, scale=2.0 / F
            )
            w2ps_b = pp.tile([P, 512], fdt, tag="ps_b")
            nc.tensor.matmul(
                w2ps_b[:, 0:488], ones_row[0:1, :], w2neg[0:1, 512:1000],
                start=True, stop=True,
            )
            nc.scalar.activation(
                w2bc[:, 512:1000], w2ps_b[:, 0:488], AF.Identity, scale=2.0 / F
            )

            # ---- Phase 2: rounds 8..15 -- x prep + gemm(r-8); tail gemms ----
            for r in range(CT, BT):
                xf = x_load(r)
                gemm(r - CT)
                x_prep(r, xf, n_act_evacs=2)
            for i in range(BT - CT, BT):
                gemm(i)

    return nc


def kernel(**inputs: np.ndarray) -> np.ndarray:
    global LAST_RESULTS
    x = np.ascontiguousarray(np.asarray(inputs["x"], dtype=np.float32))
    w = np.ascontiguousarray(np.asarray(inputs["w"], dtype=np.float32))
    assert x.shape == (B_TOTAL, F), x.shape
    assert w.shape == (C, F), w.shape

    from concourse.bass_utils import run_bass_kernel_spmd

    if "nc" not in _cache:
        nc = _build()
        _legalize_waits(nc)
        _cache["nc"] = nc
    nc = _cache["nc"]

    bs = B_TOTAL // N_CORES
    in_maps = [
        {"x": x[i * bs : (i + 1) * bs], "w": w} for i in range(N_CORES)
    ]
    res = run_bass_kernel_spmd(
        nc, in_maps, core_ids=list(range(N_CORES)),
        trace=bool(os.environ.get("BASS_TRACE")),
    )
    LAST_RESULTS = res
    return np.concatenate([r["out"] for r in res.results], axis=0)


if __name__ == "__main__":
    rng = np.random.default_rng(0)
    xs = rng.standard_normal((B_TOTAL, F), dtype=np.float32)
    ws = rng.standard_normal((C, F), dtype=np.float32) * math.sqrt(2.0 / F)
    o = kernel(x=xs, w=ws)
    print(o.shape, o.dtype, o[:2, :4])

